# revision 8
# baseline (speedup 1.0000x reference)
"""Multi-head attention kernel for 8 Trainium2 NeuronCores.

Problem: nn_MultiHeadAttention_49246095016569
  q,k,v: [S=2048, B=2, E=512] f32; per-head projections Wq/Wk/Wv [64,64],
  output FC Wfc [512,512] + bfc [512].
  The reference reshapes [S,B,E] -> [B,H,S,D] with a PLAIN reshape, so each
  (b,h) pair is a contiguous [2048,64] chunk of the flattened input.  There
  are 16 chunks; each of the 8 cores handles 2 chunks, fully independently
  (no collectives).  Output rows [512*i, 512*(i+1)) of the flattened
  [4096,512] output come from core i.

Math per chunk c (qc,kc,vc = [2048,64] slices):
  khp = kc @ g_t            (g_t = Wk.T @ Wq folds both QK projections)
  S   = qc @ khp.T          (= Q @ K.T exactly, up to fp32 rounding)
  P   = exp(S/8)            (softmax without max-subtraction; |S/8| < ~6)
  A   = (P @ (vc @ Wv.T)) / P.sum(axis=1)
  out_rows = A.reshape(256,512) @ Wfc.T + bfc

On-chip layout: everything is computed transposed (S^T = khp^T-tiles.T @ q^T)
so that softmax sums come free via a ones-column appended to V', and the FC
contraction can slice A^T directly with stride-8 access patterns.
"""

import numpy as np

import concourse.bass as bass
import concourse.mybir as mybir
import concourse.tile as tile
from concourse import bacc
from concourse import bass_utils
from concourse.masks import make_identity

F32 = mybir.dt.float32
F32R = mybir.dt.float32r

S = 2048
D = 64
E = 512
NCORES = 8
CHUNKS_PER_CORE = 2
KT = S // 128  # 16 k-tiles of 128
QB = S // 512  # 4 q-blocks of 512

# dtype for the big matmuls: f32r streams 1 row/cycle at N>=256 (vs 4 for
# plain f32).  fp32r operands must be *produced* as fp32r (the producer op
# rounds); set MM_DT = F32 to fall back to exact fp32 everywhere.
MM_DT = F32R


def build_core_program():
    nc = bacc.Bacc(trn_type="TRN2")

    q_in = nc.dram_tensor("q_in", (CHUNKS_PER_CORE * S, D), F32, kind="ExternalInput")
    k_in = nc.dram_tensor("k_in", (CHUNKS_PER_CORE * S, D), F32, kind="ExternalInput")
    v_in = nc.dram_tensor("v_in", (CHUNKS_PER_CORE * S, D), F32, kind="ExternalInput")
    g_t = nc.dram_tensor("g_t", (D, D), F32, kind="ExternalInput")
    wv_t = nc.dram_tensor("wv_t", (D, D), F32, kind="ExternalInput")
    wfc_t = nc.dram_tensor("wfc_t", (E, E), F32, kind="ExternalInput")
    bias = nc.dram_tensor("bias", (1, E), F32, kind="ExternalInput")
    out = nc.dram_tensor("out", (CHUNKS_PER_CORE * 256, E), F32, kind="ExternalOutput")

    with tile.TileContext(nc) as tc:
        with (
            tc.tile_pool(name="consts", bufs=1) as consts,
            tc.tile_pool(name="raw", bufs=2) as raw_pool,
            tc.tile_pool(name="tp", bufs=2) as tp_pool,
            tc.tile_pool(name="pt", bufs=3) as pt_pool,
            tc.tile_pool(name="at", bufs=2) as at_pool,
            tc.tile_pool(name="outp", bufs=2) as out_pool,
            tc.tile_pool(name="ps_work", bufs=3, space="PSUM") as ps_work,
            tc.tile_pool(name="ps_score", bufs=2, space="PSUM") as ps_score,
            tc.tile_pool(name="ps_acc", bufs=1, space="PSUM") as ps_acc,
            tc.tile_pool(name="ps_fc", bufs=1, space="PSUM") as ps_fc,
        ):
            identity = consts.tile([128, 128], F32)
            make_identity(nc, identity[:])

            g_sb = consts.tile([D, D], F32)
            nc.sync.dma_start(g_sb[:], g_t[:])
            wv_sb = consts.tile([D, D], F32)
            nc.sync.dma_start(wv_sb[:], wv_t[:])

            # Wfc.T packed as [128, 2048]: row (64j+d) -> partition (64j+d)%128,
            # free offset 512*(j//2).  Slice j = [64*(j%2):+64, 512*(j//2):+512]
            # Wfc.T as [64, 8, 512]: slice j = wfc_sb[:, j, :] (base partition 0)
            wfc_f32 = consts.tile([D, 8, E], F32)
            nc.sync.dma_start(
                wfc_f32[:], wfc_t[:].rearrange("(j d) e -> d j e", d=D)
            )
            if MM_DT is F32:
                wfc_sb = wfc_f32
            else:
                wfc_sb = consts.tile([D, 8, E], MM_DT, tag="wfc_r")
                nc.gpsimd.tensor_copy(wfc_sb[:], wfc_f32[:])

            bias_sb = consts.tile([1, E], F32)
            nc.sync.dma_start(bias_sb[:], bias[:])
            # broadcast bias to 128 partitions once via a K=1 outer product
            ones1 = consts.tile([1, 128], F32)
            nc.vector.memset(ones1[:], 1.0)
            bias_ps = ps_work.tile([128, E], F32, tag="work")
            nc.tensor.matmul(bias_ps[:], ones1[:], bias_sb[:], start=True, stop=True)
            bias_bc = consts.tile([128, E], F32)
            nc.vector.tensor_copy(bias_bc[:], bias_ps[:])
            ones64 = consts.tile([1, D], F32)
            nc.vector.memset(ones64[:], 1.0)
            ones_col = consts.tile([128, KT, 1], F32)
            nc.vector.memset(ones_col[:], 1.0)

            for c in range(CHUNKS_PER_CORE):
                co = c * S

                # ---- load raw chunk, layout [(p t) d -> p (t d)]: s = 16p + t
                q_raw = raw_pool.tile([128, KT * D], F32, tag="q_raw")
                k_raw = raw_pool.tile([128, KT * D], F32, tag="k_raw")
                v_raw = raw_pool.tile([128, KT * D], F32, tag="v_raw")
                nc.sync.dma_start(
                    q_raw[:], q_in[co : co + S, :].rearrange("(p t) d -> p (t d)", p=128)
                )
                nc.sync.dma_start(
                    k_raw[:], k_in[co : co + S, :].rearrange("(p t) d -> p (t d)", p=128)
                )
                nc.sync.dma_start(
                    v_raw[:], v_in[co : co + S, :].rearrange("(p t) d -> p (t d)", p=128)
                )

                # ---- PE-transpose q,k,v into [64, 2048] (column s = 16p + t)
                qhT = tp_pool.tile([D, S], MM_DT, tag="qhT")
                khT = tp_pool.tile([D, S], F32, tag="khT")
                vhT = tp_pool.tile([D, S], F32, tag="vhT")
                for name, rawt, dstT in (
                    ("q", q_raw, qhT),
                    ("k", k_raw, khT),
                    ("v", v_raw, vhT),
                ):
                    # dst column index = 16*j + t; view free dim as (j, blk, g):
                    # s = 16 j + 4 blk + g
                    dview = dstT[:].rearrange("d (j b g) -> d b g j", b=4, g=4)
                    for blk in range(4):  # 4 transposes per psum bank
                        ps_t = ps_work.tile([D, 512], F32, tag="work")
                        for g in range(4):
                            t = 4 * blk + g
                            nc.tensor.transpose(
                                ps_t[:, 128 * g : 128 * (g + 1)],
                                rawt[:, D * t : D * (t + 1)],
                                identity[:],
                            )
                        nc.vector.tensor_copy(
                            dview[:, blk],
                            ps_t[:].rearrange("d (g j) -> d g j", g=4),
                        )

                # ---- khp^T = g_t.T @ khT  (folded QK projection)
                khpT = tp_pool.tile([D, S], MM_DT, tag="khpT")
                for n in range(QB):
                    ps_p = ps_work.tile([D, 512], F32, tag="work")
                    nc.tensor.matmul(
                        ps_p[:],
                        g_sb[:],
                        khT[:, 512 * n : 512 * (n + 1)],
                        start=True,
                        stop=True,
                    )
                    nc.vector.tensor_copy(khpT[:, 512 * n : 512 * (n + 1)], ps_p[:])

                # ---- V' = vc @ Wv.T with ones column appended: [128, 17*65]
                # tile kt slice = vp[:, kt*65 : kt*65+65]
                vp = raw_pool.tile([128, KT * (D + 1)], MM_DT, tag="vp")
                vp3 = vp[:].rearrange("p (kt x) -> p kt x", x=D + 1)
                nc.vector.tensor_copy(vp3[:, :, D : D + 1], ones_col[:])
                for half in range(2):  # 8 projections of N=64 per psum bank
                    ps_v = ps_work.tile([128, 512], F32, tag="work")
                    for m in range(8):
                        kt = 8 * half + m
                        nc.tensor.matmul(
                            ps_v[:, D * m : D * (m + 1)],
                            vhT[:, 128 * kt : 128 * (kt + 1)],
                            wv_sb[:],
                            start=True,
                            stop=True,
                        )
                    nc.vector.tensor_copy(
                        vp[:].rearrange("p (kt x) -> p kt x", x=D + 1)[
                            :, 8 * half : 8 * half + 8, 0:D
                        ],
                        ps_v[:].rearrange("p (m x) -> p m x", x=D),
                    )

                # ---- attention: per q-block of 512 queries
                atT = at_pool.tile([D, S], MM_DT, tag=f"at{c}")
                for qb in range(QB):
                    qo = 512 * qb
                    pav = ps_acc.tile([D + 1, 512], F32, tag="acc")
                    for kt in range(KT):
                        st = ps_score.tile([128, 512], F32, tag="score")
                        nc.tensor.matmul(
                            st[:],
                            khpT[:, 128 * kt : 128 * (kt + 1)],
                            qhT[:, qo : qo + 512],
                            start=True,
                            stop=True,
                        )
                        ptile = pt_pool.tile([128, 512], MM_DT, tag="pt")
                        nc.scalar.activation(
                            ptile[:],
                            st[:],
                            mybir.ActivationFunctionType.Exp,
                            scale=0.125,
                        )
                        nc.tensor.matmul(
                            pav[:],
                            vp[:].rearrange("p (kt x) -> p kt x", x=D + 1)[:, kt],
                            ptile[:],
                            start=(kt == 0),
                            stop=(kt == KT - 1),
                        )
                    # normalize: A^T[:, qo:qo+512] = pav[0:64] * (1/pav[64])
                    rs = out_pool.tile([1, 512], F32, tag="rs")
                    nc.vector.reciprocal(rs[:], pav[D : D + 1, :])
                    rb_ps = ps_work.tile([D, 512], F32, tag="work")
                    nc.tensor.matmul(
                        rb_ps[:], ones64[:], rs[:], start=True, stop=True
                    )
                    rb = pt_pool.tile([D, 512], F32, tag="rb")
                    nc.scalar.activation(
                        rb[:], rb_ps[:], mybir.ActivationFunctionType.Copy
                    )
                    nc.vector.tensor_mul(atT[:, qo : qo + 512], pav[0:D, :], rb[:])

                # ---- FC: out rows rr (128 per r-tile), 8 accumulating matmuls
                atv = atT[:].rearrange("d (m r j) -> d m j r", m=2, j=8)
                for half in range(2):
                    po = ps_fc.tile([128, E], F32, tag="fc")
                    for j in range(8):
                        nc.tensor.matmul(
                            po[:],
                            atv[:, half, j, :],
                            wfc_sb[:, j, :],
                            start=(j == 0),
                            stop=(j == 7),
                        )
                    ot = out_pool.tile([128, E], F32, tag="out")
                    nc.vector.tensor_add(ot[:], po[:], bias_bc[:])
                    nc.sync.dma_start(
                        out[256 * c + 128 * half : 256 * c + 128 * (half + 1), :],
                        ot[:],
                    )

    nc.compile()
    return nc


_NC_CACHE = None


def _get_nc():
    global _NC_CACHE
    if _NC_CACHE is None:
        _NC_CACHE = build_core_program()
    return _NC_CACHE


def make_in_maps(q, k, v, Wq, Wk, Wv, Wfc, bfc):
    q = np.ascontiguousarray(q, dtype=np.float32)
    k = np.ascontiguousarray(k, dtype=np.float32)
    v = np.ascontiguousarray(v, dtype=np.float32)
    g_t = (np.asarray(Wk, np.float32).T @ np.asarray(Wq, np.float32)).astype(np.float32)
    wv_t = np.ascontiguousarray(np.asarray(Wv, np.float32).T)
    wfc_t = np.ascontiguousarray(np.asarray(Wfc, np.float32).T)
    bias = np.asarray(bfc, np.float32).reshape(1, E)

    qf = q.reshape(-1)
    kf = k.reshape(-1)
    vf = v.reshape(-1)
    C = S * D
    in_maps = []
    for i in range(NCORES):
        lo = 2 * i * C
        hi = (2 * i + 2) * C
        in_maps.append(
            dict(
                q_in=np.ascontiguousarray(qf[lo:hi].reshape(2 * S, D)),
                k_in=np.ascontiguousarray(kf[lo:hi].reshape(2 * S, D)),
                v_in=np.ascontiguousarray(vf[lo:hi].reshape(2 * S, D)),
                g_t=g_t,
                wv_t=wv_t,
                wfc_t=wfc_t,
                bias=bias,
            )
        )
    return in_maps


def kernel(q, k, v, Wq, Wk, Wv, Wfc, bfc, _trace=False):
    nc = _get_nc()
    in_maps = make_in_maps(q, k, v, Wq, Wk, Wv, Wfc, bfc)
    res = bass_utils.run_bass_kernel_spmd(
        nc, in_maps, core_ids=list(range(NCORES)), trace=_trace
    )
    out = np.concatenate([res.results[i]["out"] for i in range(NCORES)], axis=0)
    kernel.last_exec_time_ns = res.exec_time_ns
    kernel.last_results = res
    return out.reshape(S, 2, E)


# revision 9
# speedup vs baseline: 1.1130x; 1.1130x over previous
"""Multi-head attention kernel for 8 Trainium2 NeuronCores.

Problem: nn_MultiHeadAttention_49246095016569
  q,k,v: [S=2048, B=2, E=512] f32; per-head projections Wq/Wk/Wv [64,64],
  output FC Wfc [512,512] + bfc [512].
  The reference reshapes [S,B,E] -> [B,H,S,D] with a PLAIN reshape, so each
  (b,h) pair is a contiguous [2048,64] chunk of the flattened input.  There
  are 16 chunks; each of the 8 cores handles 2 chunks, fully independently
  (no collectives).  Output rows [512*i, 512*(i+1)) of the flattened
  [4096,512] output come from core i.

Math per chunk c (qc,kc,vc = [2048,64] slices):
  khp = kc @ g_t            (g_t = Wk.T @ Wq folds both QK projections)
  S   = qc @ khp.T          (= Q @ K.T exactly, up to rounding)
  P   = exp(S/8)            (softmax without max-subtraction; |S/8| < ~6)
  A   = (P @ (vc @ Wv.T)) / P.sum(axis=1)
  out_rows = A.reshape(256,512) @ Wfc.T + bfc

On-chip layout: everything is computed transposed (S^T tiles = khpT.T @ qhT)
so that softmax sums come free via a ones-column appended to V', and the FC
contraction can slice A^T directly with stride-8 access patterns.
"""

import numpy as np

import concourse.bass as bass
import concourse.mybir as mybir
import concourse.tile as tile
from concourse import bacc
from concourse import bass_utils
from concourse.masks import make_identity

F32 = mybir.dt.float32
F32R = mybir.dt.float32r
BF16 = mybir.dt.bfloat16

S = 2048
D = 64
E = 512
NCORES = 8
CHUNKS_PER_CORE = 2
KT = S // 128  # 16 k-tiles of 128
QB = S // 512  # 4 q-blocks of 512

# dtype of the streaming matmul operands.  bf16: 1 row/cycle at any clock +
# FWL weight loads.  F32R: ~2.6e-4 rel err but ran at half clock in practice.
MM_DT = BF16
ACT_EXP = mybir.ActivationFunctionType.Exp
ACT_LN = mybir.ActivationFunctionType.Ln
ACT_COPY = mybir.ActivationFunctionType.Copy


def build_core_program():
    nc = bacc.Bacc(trn_type="TRN2")

    q_in = nc.dram_tensor("q_in", (CHUNKS_PER_CORE * S, D), F32, kind="ExternalInput")
    k_in = nc.dram_tensor("k_in", (CHUNKS_PER_CORE * S, D), F32, kind="ExternalInput")
    v_in = nc.dram_tensor("v_in", (CHUNKS_PER_CORE * S, D), F32, kind="ExternalInput")
    g_t = nc.dram_tensor("g_t", (D, D), F32, kind="ExternalInput")
    wv_t = nc.dram_tensor("wv_t", (D, D), F32, kind="ExternalInput")
    wfc_t = nc.dram_tensor("wfc_t", (E, E), F32, kind="ExternalInput")
    bias = nc.dram_tensor("bias", (1, E), F32, kind="ExternalInput")
    out = nc.dram_tensor("out", (CHUNKS_PER_CORE * 256, E), F32, kind="ExternalOutput")

    with tile.TileContext(nc) as tc:
        with (
            tc.tile_pool(name="consts", bufs=1) as consts,
            tc.tile_pool(name="raw", bufs=2) as raw_pool,
            tc.tile_pool(name="tp", bufs=2) as tp_pool,
            tc.tile_pool(name="pt", bufs=4) as pt_pool,
            tc.tile_pool(name="at", bufs=2) as at_pool,
            tc.tile_pool(name="outp", bufs=2) as out_pool,
            tc.tile_pool(name="ps_work", bufs=3, space="PSUM") as ps_work,
            tc.tile_pool(name="ps_score", bufs=3, space="PSUM") as ps_score,
            tc.tile_pool(name="ps_acc", bufs=1, space="PSUM") as ps_acc,
            tc.tile_pool(name="ps_fc", bufs=1, space="PSUM") as ps_fc,
        ):
            identity = consts.tile([128, 128], F32)
            make_identity(nc, identity[:])

            g_f32 = consts.tile([D, D], F32)
            nc.sync.dma_start(g_f32[:], g_t[:])
            g_sb = consts.tile([D, D], MM_DT, tag="g_mm")
            nc.gpsimd.tensor_copy(g_sb[:], g_f32[:])
            wv_f32 = consts.tile([D, D], F32)
            nc.sync.dma_start(wv_f32[:], wv_t[:])
            wv_sb = consts.tile([D, D], MM_DT, tag="wv_mm")
            nc.gpsimd.tensor_copy(wv_sb[:], wv_f32[:])

            # Wfc.T as [64, 8, 512]: slice j = wfc_sb[:, j, :] (base partition 0)
            wfc_f32 = consts.tile([D, 8, E], F32)
            nc.sync.dma_start(
                wfc_f32[:], wfc_t[:].rearrange("(j d) e -> d j e", d=D)
            )
            wfc_sb = consts.tile([D, 8, E], MM_DT, tag="wfc_mm")
            nc.gpsimd.tensor_copy(wfc_sb[:], wfc_f32[:])

            bias_sb = consts.tile([1, E], F32)
            nc.sync.dma_start(bias_sb[:], bias[:])
            # broadcast bias to 128 partitions once via a K=1 outer product
            ones1 = consts.tile([1, 128], F32)
            nc.vector.memset(ones1[:], 1.0)
            bias_ps = ps_work.tile([128, E], F32, tag="work")
            nc.tensor.matmul(bias_ps[:], ones1[:], bias_sb[:], start=True, stop=True)
            bias_bc = consts.tile([128, E], F32)
            nc.vector.tensor_copy(bias_bc[:], bias_ps[:])
            ones64 = consts.tile([1, D], MM_DT)
            nc.vector.memset(ones64[:], 1.0)
            ones_col = consts.tile([128, KT, 1], F32)
            nc.vector.memset(ones_col[:], 1.0)

            for c in range(CHUNKS_PER_CORE):
                co = c * S

                # ---- load raw chunk, layout [(p t) d -> p (t d)]: s = 16p + t
                q_raw = raw_pool.tile([128, KT * D], F32, tag="q_raw")
                k_raw = raw_pool.tile([128, KT * D], F32, tag="k_raw")
                v_raw = raw_pool.tile([128, KT * D], F32, tag="v_raw")
                nc.sync.dma_start(
                    q_raw[:], q_in[co : co + S, :].rearrange("(p t) d -> p (t d)", p=128)
                )
                nc.sync.dma_start(
                    k_raw[:], k_in[co : co + S, :].rearrange("(p t) d -> p (t d)", p=128)
                )
                nc.sync.dma_start(
                    v_raw[:], v_in[co : co + S, :].rearrange("(p t) d -> p (t d)", p=128)
                )

                # ---- PE-transpose q,k,v into [64, 2048] MM_DT (col s = 16p+t)
                # paired: one [128,128] transpose covers s-tiles t=2g, 2g+1
                qhT = tp_pool.tile([D, S], MM_DT, tag="qhT")
                khT = tp_pool.tile([D, S], MM_DT, tag="khT")
                vhT = tp_pool.tile([D, S], MM_DT, tag="vhT")
                for rawt, dstT in ((q_raw, qhT), (k_raw, khT), (v_raw, vhT)):
                    # s = 16 j + 8 b2 + 2 gg + h
                    dv = dstT[:].rearrange(
                        "d (j b2 gg h) -> d b2 h gg j", b2=2, gg=4, h=2
                    )
                    for b2 in range(2):  # 4 paired transposes per psum bank
                        ps_t = ps_work.tile([128, 512], F32, tag="work")
                        for gg in range(4):
                            g = 4 * b2 + gg
                            nc.tensor.transpose(
                                ps_t[:, 128 * gg : 128 * (gg + 1)],
                                rawt[:, 128 * g : 128 * (g + 1)],
                                identity[:],
                            )
                        pv = ps_t[:].rearrange("x (gg j) -> x gg j", gg=4)
                        nc.vector.tensor_copy(dv[:, b2, 0], pv[0:D])
                        nc.vector.tensor_copy(dv[:, b2, 1], pv[D : 2 * D])

                # ---- khp^T = g_t.T @ khT  (folded QK projection)
                khpT = tp_pool.tile([D, S], MM_DT, tag="khpT")
                for n in range(QB):
                    ps_p = ps_work.tile([D, 512], F32, tag="work")
                    nc.tensor.matmul(
                        ps_p[:],
                        g_sb[:],
                        khT[:, 512 * n : 512 * (n + 1)],
                        start=True,
                        stop=True,
                    )
                    nc.vector.tensor_copy(khpT[:, 512 * n : 512 * (n + 1)], ps_p[:])

                # ---- V' = vc @ Wv.T with ones column: [128, 16*65] MM_DT
                vp = raw_pool.tile([128, KT * (D + 1)], MM_DT, tag="vp")
                vp3 = vp[:].rearrange("p (kt x) -> p kt x", x=D + 1)
                nc.vector.tensor_copy(vp3[:, :, D : D + 1], ones_col[:])
                for half in range(2):  # 8 projections of N=64 per psum bank
                    ps_v = ps_work.tile([128, 512], F32, tag="work")
                    for m in range(8):
                        kt = 8 * half + m
                        nc.tensor.matmul(
                            ps_v[:, D * m : D * (m + 1)],
                            vhT[:, 128 * kt : 128 * (kt + 1)],
                            wv_sb[:],
                            start=True,
                            stop=True,
                        )
                    nc.vector.tensor_copy(
                        vp3[:, 8 * half : 8 * half + 8, 0:D],
                        ps_v[:].rearrange("p (m x) -> p m x", x=D),
                    )

                # ---- attention: per q-block of 512 queries
                atT = at_pool.tile([D, S], MM_DT, tag=f"at{c}")
                for qb in range(QB):
                    qo = 512 * qb
                    pav = ps_acc.tile([D + 1, 512], F32, tag="acc")
                    for kt in range(KT):
                        st = ps_score.tile([128, 512], F32, tag="score")
                        nc.tensor.matmul(
                            st[:],
                            khpT[:, 128 * kt : 128 * (kt + 1)],
                            qhT[:, qo : qo + 512],
                            start=True,
                            stop=True,
                        )
                        ptile = pt_pool.tile([128, 512], MM_DT, tag="pt")
                        nc.scalar.activation(ptile[:], st[:], ACT_EXP, scale=0.125)
                        nc.tensor.matmul(
                            pav[:],
                            vp3[:, kt],
                            ptile[:],
                            start=(kt == 0),
                            stop=(kt == KT - 1),
                        )
                    # normalize: A^T[:, qo:+512] = pav[0:64] * (1/pav[64])
                    # 1/s = exp(-ln(s)) on ACT (DVE reciprocal on 1 partition
                    # is ~6.5 ns/elem serial -> 3.4us per q-block)
                    lns = out_pool.tile([1, 512], F32, tag="lns")
                    nc.scalar.activation(lns[:], pav[D : D + 1, :], ACT_LN)
                    rs = out_pool.tile([1, 512], MM_DT, tag="rs")
                    nc.scalar.activation(rs[:], lns[:], ACT_EXP, scale=-1.0)
                    rb_ps = ps_work.tile([D, 512], F32, tag="work")
                    nc.tensor.matmul(rb_ps[:], ones64[:], rs[:], start=True, stop=True)
                    rb = pt_pool.tile([D, 512], F32, tag="rb")
                    nc.scalar.activation(rb[:], rb_ps[:], ACT_COPY)
                    nc.vector.tensor_mul(atT[:, qo : qo + 512], pav[0:D, :], rb[:])

                # ---- FC: out rows rr (128 per r-tile), 8 accumulating matmuls
                atv = atT[:].rearrange("d (m r j) -> d m j r", m=2, j=8)
                for half in range(2):
                    po = ps_fc.tile([128, E], F32, tag="fc")
                    for j in range(8):
                        nc.tensor.matmul(
                            po[:],
                            atv[:, half, j, :],
                            wfc_sb[:, j, :],
                            start=(j == 0),
                            stop=(j == 7),
                        )
                    ot = out_pool.tile([128, E], F32, tag="out")
                    nc.vector.tensor_add(ot[:], po[:], bias_bc[:])
                    nc.sync.dma_start(
                        out[256 * c + 128 * half : 256 * c + 128 * (half + 1), :],
                        ot[:],
                    )

    nc.compile()
    return nc


_NC_CACHE = None


def _get_nc():
    global _NC_CACHE
    if _NC_CACHE is None:
        _NC_CACHE = build_core_program()
    return _NC_CACHE


def make_in_maps(q, k, v, Wq, Wk, Wv, Wfc, bfc):
    q = np.ascontiguousarray(q, dtype=np.float32)
    k = np.ascontiguousarray(k, dtype=np.float32)
    v = np.ascontiguousarray(v, dtype=np.float32)
    g_t = (np.asarray(Wk, np.float32).T @ np.asarray(Wq, np.float32)).astype(np.float32)
    wv_t = np.ascontiguousarray(np.asarray(Wv, np.float32).T)
    wfc_t = np.ascontiguousarray(np.asarray(Wfc, np.float32).T)
    bias = np.asarray(bfc, np.float32).reshape(1, E)

    qf = q.reshape(-1)
    kf = k.reshape(-1)
    vf = v.reshape(-1)
    C = S * D
    in_maps = []
    for i in range(NCORES):
        lo = 2 * i * C
        hi = (2 * i + 2) * C
        in_maps.append(
            dict(
                q_in=np.ascontiguousarray(qf[lo:hi].reshape(2 * S, D)),
                k_in=np.ascontiguousarray(kf[lo:hi].reshape(2 * S, D)),
                v_in=np.ascontiguousarray(vf[lo:hi].reshape(2 * S, D)),
                g_t=g_t,
                wv_t=wv_t,
                wfc_t=wfc_t,
                bias=bias,
            )
        )
    return in_maps


def kernel(q, k, v, Wq, Wk, Wv, Wfc, bfc, _trace=False):
    nc = _get_nc()
    in_maps = make_in_maps(q, k, v, Wq, Wk, Wv, Wfc, bfc)
    res = bass_utils.run_bass_kernel_spmd(
        nc, in_maps, core_ids=list(range(NCORES)), trace=_trace
    )
    out = np.concatenate([res.results[i]["out"] for i in range(NCORES)], axis=0)
    kernel.last_exec_time_ns = res.exec_time_ns
    kernel.last_results = res
    return out.reshape(S, 2, E)


# revision 12
# speedup vs baseline: 1.2428x; 1.1166x over previous
"""Multi-head attention kernel for 8 Trainium2 NeuronCores.

Problem: nn_MultiHeadAttention_49246095016569
  q,k,v: [S=2048, B=2, E=512] f32; per-head projections Wq/Wk/Wv [64,64],
  output FC Wfc [512,512] + bfc [512].
  The reference reshapes [S,B,E] -> [B,H,S,D] with a PLAIN reshape, so each
  (b,h) pair is a contiguous [2048,64] chunk of the flattened input.  There
  are 16 chunks; each of the 8 cores handles 2 chunks, fully independently
  (no collectives).  Output rows [512*i, 512*(i+1)) of the flattened
  [4096,512] output come from core i.

Math per chunk c (qc,kc,vc = [2048,64] slices):
  khp = kc @ g_t            (g_t = Wk.T @ Wq folds both QK projections)
  S   = qc @ khp.T          (= Q @ K.T exactly, up to rounding)
  P   = exp(S/8)            (softmax without max-subtraction; |S/8| < ~6)
  A   = (P @ (vc @ Wv.T)) / P.sum(axis=1)
  out_rows = A.reshape(256,512) @ Wfc.T + bfc

On-chip layout: everything is computed transposed (S^T tiles = khpT.T @ qhT)
so that softmax sums come free via a ones-column appended to V', and the FC
contraction can slice A^T directly with stride-8 access patterns.
"""

import numpy as np

import concourse.bass as bass
import concourse.mybir as mybir
import concourse.tile as tile
from concourse import bacc
from concourse import bass_utils
from concourse.masks import make_identity

F32 = mybir.dt.float32
F32R = mybir.dt.float32r
BF16 = mybir.dt.bfloat16

S = 2048
D = 64
E = 512
NCORES = 8
CHUNKS_PER_CORE = 2
KT = S // 128  # 16 k-tiles of 128
QB = S // 512  # 4 q-blocks of 512

# dtype of the streaming matmul operands.  bf16: 1 row/cycle at any clock +
# FWL weight loads.  F32R: ~2.6e-4 rel err but ran at half clock in practice.
MM_DT = BF16
ACT_EXP = mybir.ActivationFunctionType.Exp
ACT_LN = mybir.ActivationFunctionType.Ln
ACT_COPY = mybir.ActivationFunctionType.Copy


def build_core_program():
    nc = bacc.Bacc(trn_type="TRN2")

    q_in = nc.dram_tensor("q_in", (CHUNKS_PER_CORE * S, D), F32, kind="ExternalInput")
    k_in = nc.dram_tensor("k_in", (CHUNKS_PER_CORE * S, D), F32, kind="ExternalInput")
    v_in = nc.dram_tensor("v_in", (CHUNKS_PER_CORE * S, D), F32, kind="ExternalInput")
    g_t = nc.dram_tensor("g_t", (D, D), F32, kind="ExternalInput")
    wv_t = nc.dram_tensor("wv_t", (D, D), F32, kind="ExternalInput")
    wfc_t = nc.dram_tensor("wfc_t", (E, E), F32, kind="ExternalInput")
    bias = nc.dram_tensor("bias", (1, E), F32, kind="ExternalInput")
    out = nc.dram_tensor("out", (CHUNKS_PER_CORE * 256, E), F32, kind="ExternalOutput")

    with tile.TileContext(nc) as tc:
        with (
            tc.tile_pool(name="consts", bufs=1) as consts,
            tc.tile_pool(name="raw", bufs=2) as raw_pool,
            tc.tile_pool(name="tp", bufs=2) as tp_pool,
            tc.tile_pool(name="pt", bufs=4) as pt_pool,
            tc.tile_pool(name="at", bufs=2) as at_pool,
            tc.tile_pool(name="outp", bufs=2) as out_pool,
            tc.tile_pool(name="ps_work", bufs=3, space="PSUM") as ps_work,
            tc.tile_pool(name="ps_score", bufs=2, space="PSUM") as ps_score,
            tc.tile_pool(name="ps_acc", bufs=2, space="PSUM") as ps_acc,
            tc.tile_pool(name="ps_fc", bufs=1, space="PSUM") as ps_fc,
        ):
            identity = consts.tile([128, 128], F32)
            make_identity(nc, identity[:])

            g_f32 = consts.tile([D, D], F32)
            nc.sync.dma_start(g_f32[:], g_t[:])
            g_sb = consts.tile([D, D], MM_DT, tag="g_mm")
            nc.gpsimd.tensor_copy(g_sb[:], g_f32[:])
            wv_f32 = consts.tile([D, D], F32)
            nc.sync.dma_start(wv_f32[:], wv_t[:])
            wv_sb = consts.tile([D, D], MM_DT, tag="wv_mm")
            nc.gpsimd.tensor_copy(wv_sb[:], wv_f32[:])

            # Wfc.T as [64, 8, 512]: slice j = wfc_sb[:, j, :] (base partition 0)
            wfc_f32 = consts.tile([D, 8, E], F32)
            nc.sync.dma_start(
                wfc_f32[:], wfc_t[:].rearrange("(j d) e -> d j e", d=D)
            )
            wfc_sb = consts.tile([D, 8, E], MM_DT, tag="wfc_mm")
            nc.gpsimd.tensor_copy(wfc_sb[:], wfc_f32[:])

            bias_sb = consts.tile([1, E], F32)
            nc.sync.dma_start(bias_sb[:], bias[:])
            # broadcast bias to 128 partitions once via a K=1 outer product
            ones1 = consts.tile([1, 128], F32)
            nc.vector.memset(ones1[:], 1.0)
            bias_ps = ps_work.tile([128, E], F32, tag="work")
            nc.tensor.matmul(bias_ps[:], ones1[:], bias_sb[:], start=True, stop=True)
            bias_bc = consts.tile([128, E], F32)
            nc.vector.tensor_copy(bias_bc[:], bias_ps[:])
            ones64 = consts.tile([1, D], MM_DT)
            nc.vector.memset(ones64[:], 1.0)
            ones_col = consts.tile([128, KT, 1], F32)
            nc.vector.memset(ones_col[:], 1.0)

            for c in range(CHUNKS_PER_CORE):
                co = c * S

                # ---- load raw chunk as [128, 16, 64]: row p holds s = 128t+p
                q_raw3 = raw_pool.tile([128, KT, D], F32, tag="q_raw")
                k_raw3 = raw_pool.tile([128, KT, D], F32, tag="k_raw")
                v_raw3 = raw_pool.tile([128, KT, D], F32, tag="v_raw")
                nc.sync.dma_start(
                    q_raw3[:], q_in[co : co + S, :].rearrange("(t p) d -> p t d", p=128)
                )
                nc.sync.dma_start(
                    k_raw3[:], k_in[co : co + S, :].rearrange("(t p) d -> p t d", p=128)
                )
                nc.sync.dma_start(
                    v_raw3[:], v_in[co : co + S, :].rearrange("(t p) d -> p t d", p=128)
                )
                q_raw = q_raw3[:].rearrange("p t d -> p (t d)")
                k_raw = k_raw3[:].rearrange("p t d -> p (t d)")
                v_raw = v_raw3[:].rearrange("p t d -> p (t d)")

                # ---- PE-transpose q,k,v into [64, 2048] MM_DT (col = s)
                # paired: one [128,128] transpose covers s-tiles t=2g, 2g+1
                qhT = tp_pool.tile([D, S], MM_DT, tag="qhT")
                khT = tp_pool.tile([D, S], MM_DT, tag="khT")
                vhT = tp_pool.tile([D, S], MM_DT, tag="vhT")
                for rawt, dstT in ((q_raw, qhT), (k_raw, khT), (v_raw, vhT)):
                    # s = 1024 b2 + 256 gg + 128 h + p
                    dv = dstT[:].rearrange(
                        "d (b2 gg h p) -> d b2 h gg p", b2=2, gg=4, h=2
                    )
                    for b2 in range(2):  # 4 paired transposes per psum bank
                        ps_t = ps_work.tile([128, 512], F32, tag="work")
                        for gg in range(4):
                            g = 4 * b2 + gg
                            nc.tensor.transpose(
                                ps_t[:, 128 * gg : 128 * (gg + 1)],
                                rawt[:, 128 * g : 128 * (g + 1)],
                                identity[:],
                            )
                        pv = ps_t[:].rearrange("x (gg j) -> x gg j", gg=4)
                        nc.vector.tensor_copy(dv[:, b2, 0], pv[0:D])
                        nc.vector.tensor_copy(dv[:, b2, 1], pv[D : 2 * D])

                # ---- khp^T = g_t.T @ khT  (folded QK projection)
                khpT = tp_pool.tile([D, S], MM_DT, tag="khpT")
                for n in range(QB):
                    ps_p = ps_work.tile([D, 512], F32, tag="work")
                    nc.tensor.matmul(
                        ps_p[:],
                        g_sb[:],
                        khT[:, 512 * n : 512 * (n + 1)],
                        start=True,
                        stop=True,
                    )
                    nc.vector.tensor_copy(khpT[:, 512 * n : 512 * (n + 1)], ps_p[:])

                # ---- V' = vc @ Wv.T with ones column: [128, 16*65] MM_DT
                vp = raw_pool.tile([128, KT * (D + 1)], MM_DT, tag="vp")
                vp3 = vp[:].rearrange("p (kt x) -> p kt x", x=D + 1)
                nc.vector.tensor_copy(vp3[:, :, D : D + 1], ones_col[:])
                for half in range(2):  # 8 projections of N=64 per psum bank
                    ps_v = ps_work.tile([128, 512], F32, tag="work")
                    for m in range(8):
                        kt = 8 * half + m
                        nc.tensor.matmul(
                            ps_v[:, D * m : D * (m + 1)],
                            vhT[:, 128 * kt : 128 * (kt + 1)],
                            wv_sb[:],
                            start=True,
                            stop=True,
                        )
                    nc.vector.tensor_copy(
                        vp3[:, 8 * half : 8 * half + 8, 0:D],
                        ps_v[:].rearrange("p (m x) -> p m x", x=D),
                    )

                # ---- attention: per q-block of 512 queries
                atT = at_pool.tile([D, S], MM_DT, tag=f"at{c}")
                for qb in range(QB):
                    qo = 512 * qb
                    pav = ps_acc.tile([D + 1, 512], F32, tag="acc")
                    for kt in range(KT):
                        st = ps_score.tile([128, 512], F32, tag="score")
                        nc.tensor.matmul(
                            st[:],
                            khpT[:, 128 * kt : 128 * (kt + 1)],
                            qhT[:, qo : qo + 512],
                            start=True,
                            stop=True,
                        )
                        ptile = pt_pool.tile([128, 512], MM_DT, tag="pt")
                        nc.scalar.activation(ptile[:], st[:], ACT_EXP, scale=0.125)
                        nc.tensor.matmul(
                            pav[:],
                            vp3[:, kt],
                            ptile[:],
                            start=(kt == 0),
                            stop=(kt == KT - 1),
                        )
                    # normalize: A^T[:, qo:+512] = pav[0:64] * (1/pav[64])
                    rs = out_pool.tile([1, 512], MM_DT, tag="rs")
                    with nc.allow_low_precision(reason="softmax 1/sum in bf16"):
                        nc.vector.reciprocal(rs[:], pav[D : D + 1, :])
                    rb_ps = ps_work.tile([D, 512], F32, tag="work")
                    nc.tensor.matmul(rb_ps[:], ones64[:], rs[:], start=True, stop=True)
                    rb = pt_pool.tile([D, 512], F32, tag="rb")
                    nc.scalar.activation(rb[:], rb_ps[:], ACT_COPY)
                    nc.vector.tensor_mul(atT[:, qo : qo + 512], pav[0:D, :], rb[:])

                # ---- FC: out rows rr (128 per r-tile), 8 accumulating matmuls
                atv = atT[:].rearrange("d (m r j) -> d m j r", m=2, j=8)
                for half in range(2):
                    po = ps_fc.tile([128, E], F32, tag="fc")
                    for j in range(8):
                        nc.tensor.matmul(
                            po[:],
                            atv[:, half, j, :],
                            wfc_sb[:, j, :],
                            start=(j == 0),
                            stop=(j == 7),
                        )
                    ot = out_pool.tile([128, E], F32, tag="out")
                    nc.vector.tensor_add(ot[:], po[:], bias_bc[:])
                    nc.sync.dma_start(
                        out[256 * c + 128 * half : 256 * c + 128 * (half + 1), :],
                        ot[:],
                    )

    nc.compile()
    return nc


_NC_CACHE = None


def _get_nc():
    global _NC_CACHE
    if _NC_CACHE is None:
        _NC_CACHE = build_core_program()
    return _NC_CACHE


def make_in_maps(q, k, v, Wq, Wk, Wv, Wfc, bfc):
    q = np.ascontiguousarray(q, dtype=np.float32)
    k = np.ascontiguousarray(k, dtype=np.float32)
    v = np.ascontiguousarray(v, dtype=np.float32)
    g_t = (np.asarray(Wk, np.float32).T @ np.asarray(Wq, np.float32)).astype(np.float32)
    wv_t = np.ascontiguousarray(np.asarray(Wv, np.float32).T)
    wfc_t = np.ascontiguousarray(np.asarray(Wfc, np.float32).T)
    bias = np.asarray(bfc, np.float32).reshape(1, E)

    qf = q.reshape(-1)
    kf = k.reshape(-1)
    vf = v.reshape(-1)
    C = S * D
    in_maps = []
    for i in range(NCORES):
        lo = 2 * i * C
        hi = (2 * i + 2) * C
        in_maps.append(
            dict(
                q_in=np.ascontiguousarray(qf[lo:hi].reshape(2 * S, D)),
                k_in=np.ascontiguousarray(kf[lo:hi].reshape(2 * S, D)),
                v_in=np.ascontiguousarray(vf[lo:hi].reshape(2 * S, D)),
                g_t=g_t,
                wv_t=wv_t,
                wfc_t=wfc_t,
                bias=bias,
            )
        )
    return in_maps


def kernel(q, k, v, Wq, Wk, Wv, Wfc, bfc, _trace=False):
    nc = _get_nc()
    in_maps = make_in_maps(q, k, v, Wq, Wk, Wv, Wfc, bfc)
    res = bass_utils.run_bass_kernel_spmd(
        nc, in_maps, core_ids=list(range(NCORES)), trace=_trace
    )
    out = np.concatenate([res.results[i]["out"] for i in range(NCORES)], axis=0)
    kernel.last_exec_time_ns = res.exec_time_ns
    kernel.last_results = res
    return out.reshape(S, 2, E)
